# revision 35
# baseline (speedup 1.0000x reference)
"""Single-head attention (B=16, S=1024, D=768) on 8 Trainium2 NeuronCores.

Sharding: data-parallel over batch — each core computes 2 full batches with
all weights replicated. No collectives.

Layout strategy (all matmul operands float32r — full PE rate, ~tf32
accuracy, PE rounds raw fp32 bits internally so no rounding passes):
  - x is host-transposed to xT [d, t] so the d-contraction runs directly.
  - q, k are produced transposed ([d, t]).
  - the output projection is FOLDED into the value projection on the host
    (wf = w_out @ w_v): the kernel computes vw = x @ wf^T token-major, and
    y^T = P @ vw needs one matmul stage instead of two (P@v then @w_out^T)
    — 15% fewer FLOPs. y is produced transposed; the host transposes back.
  - S is computed TRANSPOSED ([j, i] = keys on partitions) so exp(S) lands
    directly in the layout the P-contraction needs — no transpose of P.
  - softmax denominator via a DVE pairwise add tree + gpsimd cross-partition
    all-reduce; normalization multiplies the final y^T tiles (DVE), keeping
    the reciprocal chain off the PE critical path.
  - scale 1/sqrt(D) is folded into w_q/b_q on the host; biases b_q/b_k are
    per-partition ACT bias during the PSUM->SBUF copy; b_v and b_out fold
    into b_out_eff = b_out + w_out @ b_v, applied per-partition (ACT) on
    the transposed output.
  - all large inputs are host pre-tiled so every DMA reads 128 contiguous
    per-partition blocks (minimal descriptor count, full DMA bandwidth).
"""

import sys

import numpy as np

if "/opt/trn_rl_repo" not in sys.path:
    sys.path.insert(0, "/opt/trn_rl_repo")

import concourse.bass_isa as bass_isa  # noqa: E402
import concourse.mybir as mybir  # noqa: E402
import concourse.tile as tile  # noqa: E402
from concourse import bacc  # noqa: E402
from concourse.bass_interp import get_hw_module  # noqa: E402
from concourse.bass_utils import run_bass_kernel_spmd  # noqa: E402

N_CORES = 8
B, S, D = 16, 1024, 768
BL = B // N_CORES  # batches per core
KT = D // 128  # 6 contraction tiles
F32 = mybir.dt.float32
F32R = mybir.dt.float32r

_prog = None


def _build():
    nc = bacc.Bacc("TRN2", target_bir_lowering=False, debug=False, num_devices=N_CORES)
    # pre-tiled on host: every DMA below reads 128 contiguous per-partition
    # blocks (minimal descriptor count, full DMA bandwidth)
    xT_d = nc.dram_tensor("xTt", [BL, 2, 2, 128, 3, 512], F32R,
                          kind="ExternalInput").ap()
    wqk_d = nc.dram_tensor("wqkt", [3, 128, KT, 256], F32R,
                           kind="ExternalInput").ap()
    onc_d = nc.dram_tensor("onesc", [128, 1], F32R, kind="ExternalInput").ap()
    wf_d = nc.dram_tensor("wft", [2, 128, KT, 386], F32R,
                          kind="ExternalInput").ap()
    boute_d = nc.dram_tensor("boute", [128, KT], F32, kind="ExternalInput").ap()
    warm_d = nc.dram_tensor("warm", [128, 128], F32R, kind="ExternalInput").ap()
    y_d = nc.dram_tensor("y", [BL, D, S], F32, kind="ExternalOutput").ap()

    Exp = mybir.ActivationFunctionType.Exp
    Ident = mybir.ActivationFunctionType.Identity
    Mult = mybir.AluOpType.mult
    Add = mybir.AluOpType.add

    with tile.TileContext(nc) as tc:
        with tc.tile_pool(name="consts", bufs=1) as consts, \
             tc.tile_pool(name="wqk", bufs=3) as wqkp, \
             tc.tile_pool(name="wf", bufs=2) as wfp, \
             tc.tile_pool(name="xT", bufs=4) as xp, \
             tc.tile_pool(name="qk", bufs=1) as qkp, \
             tc.tile_pool(name="vw", bufs=1) as vwp, \
             tc.tile_pool(name="pt", bufs=3) as ptp, \
             tc.tile_pool(name="y", bufs=3) as yp, \
             tc.tile_pool(name="small", bufs=1) as smallp, \
             tc.tile_pool(name="mm", bufs=8, space="PSUM") as mmp:

            boute_sb = consts.tile([128, KT], F32)
            onc_sb = consts.tile([128, 1], F32R)

            # PE warmup: dummy matmuls during the initial DMA wait open the
            # HAM clock gate (cold PE runs at 1.2 GHz for ~3.4us otherwise)
            warm_sb = consts.tile([128, 128], F32R)
            nc.sync.dma_start(warm_sb[:], warm_d[:])
            wps = mmp.tile([128, 512], F32, tag="mm", name="warmps")
            for i in range(24):
                nc.tensor.matmul(wps[:, :128], warm_sb[:], warm_sb[:],
                                 start=True, stop=True)

            for b in range(BL):
                # first weight slice before the big x DMAs so the PE can
                # start as soon as x-half 0 lands
                # input DMAs in exact consumption order for the th-outer
                # A-Z loop: wsl0 cols 0:128, x half 0, wsl0 cols 128:, wsl1,
                # wsl2 (all six th=0 chains), then x half 1, consts
                wsls = [wqkp.tile([128, KT, 256], F32R, tag="wqk",
                                  name=f"wsl{b}_{i}") for i in range(3)]
                nc.sync.dma_start(wsls[0][:, :, :128], wqk_d[0, :, :, :128])
                xh = []
                for h in range(2):
                    t = xp.tile([128, KT, 512], F32R, tag="xT")
                    xh.append(t)
                for kc in range(2):
                    nc.sync.dma_start(xh[0][:, 3 * kc:3 * (kc + 1)],
                                      xT_d[b, 0, kc])
                nc.sync.dma_start(wsls[0][:, :, 128:], wqk_d[0, :, :, 128:])
                nc.sync.dma_start(wsls[1][:], wqk_d[1])
                nc.sync.dma_start(wsls[2][:], wqk_d[2])
                for kc in range(2):
                    nc.sync.dma_start(xh[1][:, 3 * kc:3 * (kc + 1)],
                                      xT_d[b, 1, kc])
                if b == 0:
                    # needed only late (last-tile denominator / final bias)
                    nc.sync.dma_start(onc_sb[:], onc_d[:])
                    nc.sync.dma_start(boute_sb[:], boute_d[:])
                ZT = qkp.tile([128, KT, S], F32R, tag="ZT")
                vw_sb = vwp.tile([128, 8, D + 2], F32R, tag="vw")

                # A-Z: Z^T[e, t] for Z = x @ (s*w_q^T @ w_k); S = Z @ x^T.
                # th-outer: the six x-half-0 chains run while x half 1 is
                # still in flight (weights alone gate them)
                for th in range(2):
                    for ew in range(3):  # weight slices of 256 e-columns
                        wsl = wsls[ew]
                        for eh in range(2):  # 128-wide halves of the slice
                            et = 2 * ew + eh
                            ps = mmp.tile([128, 512], F32, tag="mm")
                            for kt in range(KT):
                                nc.tensor.matmul(ps[:], wsl[:, kt, 128 * eh:128 * (eh + 1)],
                                                 xh[th][:, kt],
                                                 start=(kt == 0), stop=(kt == KT - 1))
                            nc.scalar.copy(ZT[:, et, 512 * th:512 * (th + 1)], ps[:])

                # A-vw: vw[t, e] = x @ [wf | u]^T token-major. Column 768 is
                # colterm[t] = x @ u — the surviving softmax bias, emerging in
                # exactly the per-partition layout the exp ACT bias needs.
                for f2, (foff, fsz) in enumerate(((0, 384), (384, 386))):
                    wf = wfp.tile([128, KT, 386], F32R, tag="wf")
                    nc.sync.dma_start(wf[:, :, :fsz], wf_d[f2, :, :, :fsz])
                    for tt in range(8):
                        ps = mmp.tile([128, 512], F32, tag="mm")
                        for kt in range(KT):
                            nc.tensor.matmul(ps[:, :fsz],
                                             xh[tt // 4][:, kt, 128 * (tt % 4):128 * (tt % 4 + 1)],
                                             wf[:, kt, :fsz],
                                             start=(kt == 0), stop=(kt == KT - 1))
                        nc.vector.tensor_copy(vw_sb[:, tt, foff:foff + fsz], ps[:, :fsz])

                for ih in range(2):
                    # B: S^T[j, i] tiles -> exp -> PT (unnormalized)
                    PT = ptp.tile([128, 8, 512], F32R, tag="PT")
                    for jt in range(8):
                        ps = mmp.tile([128, 512], F32, tag="mm")
                        for dt in range(KT):
                            nc.tensor.matmul(ps[:], xh[jt // 4][:, dt, 128 * (jt % 4):128 * (jt % 4 + 1)],
                                             ZT[:, dt, 512 * ih:512 * (ih + 1)],
                                             start=(dt == 0), stop=(dt == KT - 1))
                        nc.scalar.activation(PT[:, jt], ps[:], Exp,
                                             bias=vw_sb[:, jt, D:D + 1])

                    # C: softmax denominator. Mid-kernel the DVE add tree +
                    # gpsimd all-reduce hides behind PE work; for the very
                    # last tile there is no PE work left to hide behind, so
                    # use PE row-sum matmuls (ready right after the last exp)
                    rb = smallp.tile([128, 512], F32, tag="rb")
                    if b == BL - 1 and ih == 1:
                        pr = mmp.tile([128, 512], F32, tag="mm", name="sumrow")
                        for jt in range(8):
                            nc.tensor.matmul(pr[0:1, :], onc_sb[:], PT[:, jt],
                                             start=(jt == 0), stop=(jt == 7))
                        rrow = smallp.tile([1, 512], F32, tag="rrow")
                        nc.vector.reciprocal_approx_fast(rrow[0:1, :], pr[0:1, :])
                        nc.gpsimd.partition_broadcast(rb[:], rrow[0:1, :])
                    else:
                        tree = smallp.tile([128, 4, 512], F32, tag="tree")
                        for p in range(4):
                            nc.vector.tensor_tensor(tree[:, p], PT[:, 2 * p],
                                                    PT[:, 2 * p + 1], Add)
                        nc.vector.tensor_tensor(tree[:, 0], tree[:, 0], tree[:, 1], Add)
                        nc.vector.tensor_tensor(tree[:, 2], tree[:, 2], tree[:, 3], Add)
                        nc.vector.tensor_tensor(tree[:, 1], tree[:, 0], tree[:, 2], Add)
                        nc.gpsimd.partition_all_reduce(tree[:, 3], tree[:, 1], 128,
                                                       bass_isa.ReduceOp.add)
                        nc.vector.reciprocal_approx_fast(rb[:], tree[:, 3])

                    # D: y^T[e, i] = (vw^T @ P^T) * (1/denom) + b_out_eff
                    for et in range(KT):
                        ps = mmp.tile([128, 512], F32, tag="mm")
                        for jt in range(8):
                            nc.tensor.matmul(ps[:], vw_sb[:, jt, 128 * et:128 * (et + 1)],
                                             PT[:, jt], start=(jt == 0), stop=(jt == 7))
                        yt = yp.tile([128, 512], F32, tag="y")
                        if b == BL - 1 and ih == 1 and et == KT - 1:
                            # very last tile: halve the post-chain so the
                            # final DMA starts sooner (shorter drain tail)
                            for ho in (0, 256):
                                sl = slice(ho, ho + 256)
                                nc.vector.tensor_tensor(yt[:, sl], ps[:, sl],
                                                        rb[:, sl], Mult)
                                nc.vector.tensor_scalar_add(yt[:, sl], yt[:, sl],
                                                            boute_sb[:, et:et + 1])
                                nc.scalar.dma_start(
                                    y_d[b, 128 * et:128 * (et + 1),
                                        512 * ih + ho:512 * ih + ho + 256],
                                    yt[:, sl])
                        else:
                            nc.vector.tensor_tensor(yt[:], ps[:], rb[:], Mult)
                            nc.vector.tensor_scalar_add(yt[:], yt[:],
                                                        boute_sb[:, et:et + 1])
                            nc.scalar.dma_start(
                                y_d[b, 128 * et:128 * (et + 1),
                                    512 * ih:512 * (ih + 1)],
                                yt[:])

    nc.compile()
    nc.m = get_hw_module(nc.m)
    return nc


def _prepare_in_maps(x, w_qkv, b_qkv, w_out, b_out):
    x = np.asarray(x, dtype=np.float32)
    w_qkv = np.asarray(w_qkv, dtype=np.float32)
    b_qkv = np.asarray(b_qkv, dtype=np.float32)
    w_out = np.asarray(w_out, dtype=np.float32)
    b_out = np.asarray(b_out, dtype=np.float32)

    s = D ** -0.5
    w_q = w_qkv[:D, :]
    w_k = w_qkv[D:2 * D, :]
    w_v = w_qkv[2 * D:, :]
    # folded score projection: S = x @ wqkf @ x^T with wqkf = s*w_q^T @ w_k
    wqkf = (s * w_q.T) @ w_k  # [d_in, d_out]
    # only surviving score bias: colterm = x @ u, u = w_k^T @ (s*b_q)
    u = w_k.T @ (s * b_qkv[:D])  # [D]
    # folded value/output projection, augmented with u as a 769th column so
    # colterm falls out of the vw matmul for free
    wf = w_out @ w_v  # [D, D]
    # u as column 768; column 769 zero-pads to an even fp32r free-dim
    wf_aug = np.concatenate(
        [wf.T, u[:, None], np.zeros((D, 1), np.float32)], axis=1)  # [d, D+2]
    b_out_eff = (b_out + w_out @ b_qkv[2 * D:]).astype(np.float32)
    boute_arr = np.ascontiguousarray(b_out_eff.reshape(KT, 128).T)  # [128, KT]
    # pre-tiled weights: [slice, partition, ko, cols] with contiguous cols
    wqk_t = np.ascontiguousarray(
        wqkf.reshape(KT, 128, 3, 256).transpose(2, 1, 0, 3).astype(np.float32))
    wf_t = np.zeros((2, 128, KT, 386), np.float32)
    for f2, (foff, fsz) in enumerate(((0, 384), (384, 386))):
        wf_t[f2, :, :, :fsz] = wf_aug[:, foff:foff + fsz].reshape(
            KT, 128, fsz).transpose(1, 0, 2)

    in_maps = []
    for c in range(N_CORES):
        xl = x[BL * c:BL * (c + 1)]
        xT = xl.transpose(0, 2, 1)  # [BL, D, S]
        # [BL, h, kc, p, k3, t] with contiguous [k3, t] per partition
        xT_t = np.ascontiguousarray(
            xT.reshape(BL, 2, 3, 128, 2, 512).transpose(0, 4, 1, 3, 2, 5))
        in_maps.append({
            "xTt": xT_t, "wqkt": wqk_t, "wft": wf_t,
            "onesc": np.ones((128, 1), np.float32),
            "boute": boute_arr,
            "warm": np.ones((128, 128), np.float32),
        })
    return in_maps


def _get_prog():
    global _prog
    if _prog is None:
        _prog = _build()
    return _prog


def _run(in_maps, **kwargs):
    res = run_bass_kernel_spmd(_get_prog(), in_maps, list(range(N_CORES)), **kwargs)
    return res


def kernel(x, w_qkv, b_qkv, w_out, b_out):
    in_maps = _prepare_in_maps(x, w_qkv, b_qkv, w_out, b_out)
    res = _run(in_maps)
    # kernel produces y transposed ([BL, D, S]); transpose back on host
    y = np.concatenate(
        [res.results[c]["y"].transpose(0, 2, 1) for c in range(N_CORES)], axis=0)
    return np.ascontiguousarray(y).astype(np.float32)


# revision 36
# speedup vs baseline: 1.0019x; 1.0019x over previous
"""Single-head attention (B=16, S=1024, D=768) on 8 Trainium2 NeuronCores.

Sharding: data-parallel over batch — each core computes 2 full batches with
all weights replicated. No collectives.

Layout strategy (all matmul operands float32r — full PE rate, ~tf32
accuracy, PE rounds raw fp32 bits internally so no rounding passes):
  - x is host-transposed to xT [d, t] so the d-contraction runs directly.
  - q, k are produced transposed ([d, t]).
  - the output projection is FOLDED into the value projection on the host
    (wf = w_out @ w_v): the kernel computes vw = x @ wf^T token-major, and
    y^T = P @ vw needs one matmul stage instead of two (P@v then @w_out^T)
    — 15% fewer FLOPs. y is produced transposed; the host transposes back.
  - S is computed TRANSPOSED ([j, i] = keys on partitions) so exp(S) lands
    directly in the layout the P-contraction needs — no transpose of P.
  - softmax denominator via a DVE pairwise add tree + gpsimd cross-partition
    all-reduce; normalization multiplies the final y^T tiles (DVE), keeping
    the reciprocal chain off the PE critical path.
  - scale 1/sqrt(D) is folded into w_q/b_q on the host; biases b_q/b_k are
    per-partition ACT bias during the PSUM->SBUF copy; b_v and b_out fold
    into b_out_eff = b_out + w_out @ b_v, applied per-partition (ACT) on
    the transposed output.
  - all large inputs are host pre-tiled so every DMA reads 128 contiguous
    per-partition blocks (minimal descriptor count, full DMA bandwidth).
"""

import sys

import numpy as np

if "/opt/trn_rl_repo" not in sys.path:
    sys.path.insert(0, "/opt/trn_rl_repo")

import concourse.bass_isa as bass_isa  # noqa: E402
import concourse.mybir as mybir  # noqa: E402
import concourse.tile as tile  # noqa: E402
from concourse import bacc  # noqa: E402
from concourse.bass_interp import get_hw_module  # noqa: E402
from concourse.bass_utils import run_bass_kernel_spmd  # noqa: E402

N_CORES = 8
B, S, D = 16, 1024, 768
BL = B // N_CORES  # batches per core
KT = D // 128  # 6 contraction tiles
F32 = mybir.dt.float32
F32R = mybir.dt.float32r

_prog = None


def _build():
    nc = bacc.Bacc("TRN2", target_bir_lowering=False, debug=False, num_devices=N_CORES)
    # pre-tiled on host: every DMA below reads 128 contiguous per-partition
    # blocks (minimal descriptor count, full DMA bandwidth)
    xT_d = nc.dram_tensor("xTt", [BL, 2, 2, 128, 3, 512], F32R,
                          kind="ExternalInput").ap()
    wqk_d = nc.dram_tensor("wqkt", [3, 128, KT, 256], F32R,
                           kind="ExternalInput").ap()
    onc_d = nc.dram_tensor("onesc", [128, 1], F32R, kind="ExternalInput").ap()
    wf_d = nc.dram_tensor("wft", [2, 128, KT, 386], F32R,
                          kind="ExternalInput").ap()
    boute_d = nc.dram_tensor("boute", [128, KT], F32, kind="ExternalInput").ap()
    warm_d = nc.dram_tensor("warm", [128, 128], F32R, kind="ExternalInput").ap()
    y_d = nc.dram_tensor("y", [BL, D, S], F32, kind="ExternalOutput").ap()

    Exp = mybir.ActivationFunctionType.Exp
    Ident = mybir.ActivationFunctionType.Identity
    Mult = mybir.AluOpType.mult
    Add = mybir.AluOpType.add

    with tile.TileContext(nc) as tc:
        with tc.tile_pool(name="consts", bufs=1) as consts, \
             tc.tile_pool(name="wqk", bufs=3) as wqkp, \
             tc.tile_pool(name="wf", bufs=2) as wfp, \
             tc.tile_pool(name="xT", bufs=3) as xp, \
             tc.tile_pool(name="qk", bufs=1) as qkp, \
             tc.tile_pool(name="vw", bufs=1) as vwp, \
             tc.tile_pool(name="pt", bufs=2) as ptp, \
             tc.tile_pool(name="y", bufs=3) as yp, \
             tc.tile_pool(name="small", bufs=1) as smallp, \
             tc.tile_pool(name="mm", bufs=8, space="PSUM") as mmp:

            boute_sb = consts.tile([128, KT], F32)
            onc_sb = consts.tile([128, 1], F32R)

            # PE warmup: dummy matmuls during the initial DMA wait open the
            # HAM clock gate (cold PE runs at 1.2 GHz for ~3.4us otherwise)
            warm_sb = consts.tile([128, 128], F32R)
            nc.sync.dma_start(warm_sb[:], warm_d[:])
            wps = mmp.tile([128, 512], F32, tag="mm", name="warmps")
            for i in range(24):
                nc.tensor.matmul(wps[:, :128], warm_sb[:], warm_sb[:],
                                 start=True, stop=True)

            for b in range(BL):
                # first weight slice before the big x DMAs so the PE can
                # start as soon as x-half 0 lands
                # input DMAs in exact consumption order for the th-outer
                # A-Z loop: wsl0 cols 0:128, x half 0, wsl0 cols 128:, wsl1,
                # wsl2 (all six th=0 chains), then x half 1, consts
                wsls = [wqkp.tile([128, KT, 256], F32R, tag="wqk",
                                  name=f"wsl{b}_{i}") for i in range(3)]
                nc.sync.dma_start(wsls[0][:, :, :128], wqk_d[0, :, :, :128])
                xh = []
                for h in range(2):
                    t = xp.tile([128, KT, 512], F32R, tag="xT")
                    xh.append(t)
                for kc in range(2):
                    nc.sync.dma_start(xh[0][:, 3 * kc:3 * (kc + 1)],
                                      xT_d[b, 0, kc])
                nc.sync.dma_start(wsls[0][:, :, 128:], wqk_d[0, :, :, 128:])
                nc.sync.dma_start(wsls[1][:], wqk_d[1])
                nc.sync.dma_start(wsls[2][:], wqk_d[2])
                for kc in range(2):
                    nc.sync.dma_start(xh[1][:, 3 * kc:3 * (kc + 1)],
                                      xT_d[b, 1, kc])
                if b == 0:
                    # needed only late (last-tile denominator / final bias)
                    nc.sync.dma_start(onc_sb[:], onc_d[:])
                    nc.sync.dma_start(boute_sb[:], boute_d[:])
                ZT = qkp.tile([128, KT, S], F32R, tag="ZT")
                vw_sb = vwp.tile([128, 8, D + 2], F32R, tag="vw")

                # A-Z: Z^T[e, t] for Z = x @ (s*w_q^T @ w_k); S = Z @ x^T.
                # th-outer: the six x-half-0 chains run while x half 1 is
                # still in flight (weights alone gate them)
                for th in range(2):
                    for ew in range(3):  # weight slices of 256 e-columns
                        wsl = wsls[ew]
                        for eh in range(2):  # 128-wide halves of the slice
                            et = 2 * ew + eh
                            ps = mmp.tile([128, 512], F32, tag="mm")
                            for kt in range(KT):
                                nc.tensor.matmul(ps[:], wsl[:, kt, 128 * eh:128 * (eh + 1)],
                                                 xh[th][:, kt],
                                                 start=(kt == 0), stop=(kt == KT - 1))
                            nc.scalar.copy(ZT[:, et, 512 * th:512 * (th + 1)], ps[:])

                # A-vw: vw[t, e] = x @ [wf | u]^T token-major. Column 768 is
                # colterm[t] = x @ u — the surviving softmax bias, emerging in
                # exactly the per-partition layout the exp ACT bias needs.
                for f2, (foff, fsz) in enumerate(((0, 384), (384, 386))):
                    wf = wfp.tile([128, KT, 386], F32R, tag="wf")
                    nc.sync.dma_start(wf[:, :, :fsz], wf_d[f2, :, :, :fsz])
                    for tt in range(8):
                        ps = mmp.tile([128, 512], F32, tag="mm")
                        for kt in range(KT):
                            nc.tensor.matmul(ps[:, :fsz],
                                             xh[tt // 4][:, kt, 128 * (tt % 4):128 * (tt % 4 + 1)],
                                             wf[:, kt, :fsz],
                                             start=(kt == 0), stop=(kt == KT - 1))
                        nc.vector.tensor_copy(vw_sb[:, tt, foff:foff + fsz], ps[:, :fsz])

                for ih in range(2):
                    # B: S^T[j, i] tiles -> exp -> PT (unnormalized)
                    PT = ptp.tile([128, 8, 512], F32R, tag="PT")
                    for jt in range(8):
                        ps = mmp.tile([128, 512], F32, tag="mm")
                        for dt in range(KT):
                            nc.tensor.matmul(ps[:], xh[jt // 4][:, dt, 128 * (jt % 4):128 * (jt % 4 + 1)],
                                             ZT[:, dt, 512 * ih:512 * (ih + 1)],
                                             start=(dt == 0), stop=(dt == KT - 1))
                        nc.scalar.activation(PT[:, jt], ps[:], Exp,
                                             bias=vw_sb[:, jt, D:D + 1])

                    # C: softmax denominator. Mid-kernel the DVE add tree +
                    # gpsimd all-reduce hides behind PE work; for the very
                    # last tile there is no PE work left to hide behind, so
                    # use PE row-sum matmuls (ready right after the last exp)
                    rb = smallp.tile([128, 512], F32, tag="rb")
                    if b == BL - 1 and ih == 1:
                        pr = mmp.tile([128, 512], F32, tag="mm", name="sumrow")
                        for jt in range(8):
                            nc.tensor.matmul(pr[0:1, :], onc_sb[:], PT[:, jt],
                                             start=(jt == 0), stop=(jt == 7))
                        rrow = smallp.tile([1, 512], F32, tag="rrow")
                        nc.vector.reciprocal_approx_fast(rrow[0:1, :], pr[0:1, :])
                        nc.gpsimd.partition_broadcast(rb[:], rrow[0:1, :])
                    else:
                        tree = smallp.tile([128, 4, 512], F32, tag="tree")
                        for p in range(4):
                            nc.vector.tensor_tensor(tree[:, p], PT[:, 2 * p],
                                                    PT[:, 2 * p + 1], Add)
                        nc.vector.tensor_tensor(tree[:, 0], tree[:, 0], tree[:, 1], Add)
                        nc.vector.tensor_tensor(tree[:, 2], tree[:, 2], tree[:, 3], Add)
                        nc.vector.tensor_tensor(tree[:, 1], tree[:, 0], tree[:, 2], Add)
                        nc.gpsimd.partition_all_reduce(tree[:, 3], tree[:, 1], 128,
                                                       bass_isa.ReduceOp.add)
                        nc.vector.reciprocal_approx_fast(rb[:], tree[:, 3])

                    # D: y^T[e, i] = (vw^T @ P^T) * (1/denom) + b_out_eff
                    for et in range(KT):
                        ps = mmp.tile([128, 512], F32, tag="mm")
                        for jt in range(8):
                            nc.tensor.matmul(ps[:], vw_sb[:, jt, 128 * et:128 * (et + 1)],
                                             PT[:, jt], start=(jt == 0), stop=(jt == 7))
                        yt = yp.tile([128, 512], F32, tag="y")
                        if b == BL - 1 and ih == 1 and et == KT - 1:
                            # very last tile: halve the post-chain so the
                            # final DMA starts sooner (shorter drain tail)
                            for ho in (0, 256):
                                sl = slice(ho, ho + 256)
                                nc.vector.tensor_tensor(yt[:, sl], ps[:, sl],
                                                        rb[:, sl], Mult)
                                nc.vector.tensor_scalar_add(yt[:, sl], yt[:, sl],
                                                            boute_sb[:, et:et + 1])
                                nc.scalar.dma_start(
                                    y_d[b, 128 * et:128 * (et + 1),
                                        512 * ih + ho:512 * ih + ho + 256],
                                    yt[:, sl])
                        else:
                            nc.vector.tensor_tensor(yt[:], ps[:], rb[:], Mult)
                            nc.vector.tensor_scalar_add(yt[:], yt[:],
                                                        boute_sb[:, et:et + 1])
                            nc.scalar.dma_start(
                                y_d[b, 128 * et:128 * (et + 1),
                                    512 * ih:512 * (ih + 1)],
                                yt[:])

    nc.compile()
    nc.m = get_hw_module(nc.m)
    return nc


def _prepare_in_maps(x, w_qkv, b_qkv, w_out, b_out):
    x = np.asarray(x, dtype=np.float32)
    w_qkv = np.asarray(w_qkv, dtype=np.float32)
    b_qkv = np.asarray(b_qkv, dtype=np.float32)
    w_out = np.asarray(w_out, dtype=np.float32)
    b_out = np.asarray(b_out, dtype=np.float32)

    s = D ** -0.5
    w_q = w_qkv[:D, :]
    w_k = w_qkv[D:2 * D, :]
    w_v = w_qkv[2 * D:, :]
    # folded score projection: S = x @ wqkf @ x^T with wqkf = s*w_q^T @ w_k
    wqkf = (s * w_q.T) @ w_k  # [d_in, d_out]
    # only surviving score bias: colterm = x @ u, u = w_k^T @ (s*b_q)
    u = w_k.T @ (s * b_qkv[:D])  # [D]
    # folded value/output projection, augmented with u as a 769th column so
    # colterm falls out of the vw matmul for free
    wf = w_out @ w_v  # [D, D]
    # u as column 768; column 769 zero-pads to an even fp32r free-dim
    wf_aug = np.concatenate(
        [wf.T, u[:, None], np.zeros((D, 1), np.float32)], axis=1)  # [d, D+2]
    b_out_eff = (b_out + w_out @ b_qkv[2 * D:]).astype(np.float32)
    boute_arr = np.ascontiguousarray(b_out_eff.reshape(KT, 128).T)  # [128, KT]
    # pre-tiled weights: [slice, partition, ko, cols] with contiguous cols
    wqk_t = np.ascontiguousarray(
        wqkf.reshape(KT, 128, 3, 256).transpose(2, 1, 0, 3).astype(np.float32))
    wf_t = np.zeros((2, 128, KT, 386), np.float32)
    for f2, (foff, fsz) in enumerate(((0, 384), (384, 386))):
        wf_t[f2, :, :, :fsz] = wf_aug[:, foff:foff + fsz].reshape(
            KT, 128, fsz).transpose(1, 0, 2)

    in_maps = []
    for c in range(N_CORES):
        xl = x[BL * c:BL * (c + 1)]
        xT = xl.transpose(0, 2, 1)  # [BL, D, S]
        # [BL, h, kc, p, k3, t] with contiguous [k3, t] per partition
        xT_t = np.ascontiguousarray(
            xT.reshape(BL, 2, 3, 128, 2, 512).transpose(0, 4, 1, 3, 2, 5))
        in_maps.append({
            "xTt": xT_t, "wqkt": wqk_t, "wft": wf_t,
            "onesc": np.ones((128, 1), np.float32),
            "boute": boute_arr,
            "warm": np.ones((128, 128), np.float32),
        })
    return in_maps


def _get_prog():
    global _prog
    if _prog is None:
        _prog = _build()
    return _prog


def _run(in_maps, **kwargs):
    res = run_bass_kernel_spmd(_get_prog(), in_maps, list(range(N_CORES)), **kwargs)
    return res


def kernel(x, w_qkv, b_qkv, w_out, b_out):
    in_maps = _prepare_in_maps(x, w_qkv, b_qkv, w_out, b_out)
    res = _run(in_maps)
    # kernel produces y transposed ([BL, D, S]); transpose back on host
    y = np.concatenate(
        [res.results[c]["y"].transpose(0, 2, 1) for c in range(N_CORES)], axis=0)
    return np.ascontiguousarray(y).astype(np.float32)
